# revision 13
# baseline (speedup 1.0000x reference)
"""Trainium2 Bass kernel for nn_LRSVConv (low-rank spatially-varying conv).

Computes, for full inputs
    x            [8, 32, 256, 256]  f32
    conv_w       [192, 32, 3, 3]    f32   (192 = RANK(3) * C_OUT(64))
    kernel_weight[2, 256, 256]      f32
the reference:
    y   = conv2d(x, conv_w, stride 1, pad 1)      # [8, 192, 256, 256]
    y   = y.reshape(8, 3, 64, 256, 256)
    out = y[:,0] + kw[0]*y[:,1] + kw[1]*y[:,2]    # [8, 64, 256, 256]

Sharding: spatial (H) bands of 32 output rows across the 8 cores, all
batches per core, so the per-pixel blend weights are loaded once per core.

Per core, per supertile (4 output rows = 2 blocks q of 512 px):
  - conv matmuls in bf16 (FWL-able weight loads, exact-enough: measured
    rel err 2.4e-3 vs the 2e-2 gate), K=96 (3 kh x 32 c_in), kw via
    free-dim shifts of a 3x-replicated imcol tile:
      per kw: bc[q0] (M=128: ranks 1|2), bc[q1], and a col-tiled
      concurrent pair a2[0:64] / a2[64:128] (M=64: rank 0, blocks q0/q1)
    -> 9 matmul time-slots instead of 12.
  - blend: m = bc * svb on DVE (svb = host-broadcast sv weights), then a
    col-tiled pair of identity matmuls folds m's two 64-row halves into
    a2 -> out = y0 + sv1*y1 + sv2*y2 in PSUM; ACT evacuates to SBUF.
  - blend stage runs one supertile behind the conv stage so the PE never
    waits for DVE.
  - imcol for batch b+1 is prefetched (single 3-window DMA) at the start
    of batch b so batch boundaries don't stall the PE.
"""

import numpy as np
import ml_dtypes

B, C_IN, C_OUT, RANK, IMG = 8, 32, 64, 3, 256
N_CORES = 8
BAND = IMG // N_CORES          # 32 output rows per core
WP = IMG + 2                   # padded width 258
ROWS_IN = BAND + 2             # input rows needed per band (with halo)
SUPER = 8                      # supertiles per (batch, band): 4 rows each
SROWS = BAND // SUPER          # 4 image rows per supertile
NBLK = 512                     # pixels per matmul block (2 image rows)

_F32 = np.float32
_BF16 = ml_dtypes.bfloat16


def _build_bass():
    import concourse.mybir as mybir
    import concourse.tile as tile
    from concourse import bacc

    f32 = mybir.dt.float32
    f32r = mybir.dt.float32r
    bf16 = mybir.dt.bfloat16
    nc = bacc.Bacc("TRN2", target_bir_lowering=False, debug=False)

    xs_t = nc.dram_tensor("xs", (B, C_IN, ROWS_IN * WP), bf16, kind="ExternalInput")
    # wtbc[kw]: [96, 128] ranks (1|2); wta[kw]: [96, 64] rank 0
    wtbc_t = nc.dram_tensor("wtbc", (3, 96, 128), bf16, kind="ExternalInput")
    wta_t = nc.dram_tensor("wta", (3, 96, 64), bf16, kind="ExternalInput")
    # svb rows 0:64 = sv1 (bcast over c), rows 64:128 = sv2; cols = (t, q, j)
    svb_t = nc.dram_tensor("svb", (128, SUPER * 2 * NBLK), bf16, kind="ExternalInput")
    # ident: [128, 64] = [I64; I64] (fold the two 64-row halves)
    id_t = nc.dram_tensor("ident", (128, 64), bf16, kind="ExternalInput")
    out_t = nc.dram_tensor("out", (B, C_OUT, BAND, IMG), f32, kind="ExternalOutput")

    out_r = out_t.ap().rearrange(
        "b c (t q r) w -> b q c t (r w)", t=SUPER, q=2, r=SROWS // 2
    )

    with tile.TileContext(nc) as tc:
        with (
            tc.tile_pool(name="const", bufs=1) as cpool,
            tc.tile_pool(name="imcol", bufs=3) as ipool,
            tc.tile_pool(name="psum", bufs=2, space="PSUM") as ppool,
            tc.tile_pool(name="tmp", bufs=3) as tpool,
            tc.tile_pool(name="outp", bufs=4) as opool,
        ):
            # consts go on the ACT HWDGE ring so they don't serialize in
            # front of the imcol loads on the sync ring; svb (2 MB) is
            # loaded in per-supertile chunks so blend(0) isn't gated on
            # the whole tensor.
            wtbc_sb = cpool.tile([96, 3, 128], bf16)
            nc.scalar.dma_start(wtbc_sb[:], wtbc_t.ap().rearrange("k p m -> p k m"))
            wta_sb = cpool.tile([96, 3, 64], bf16)
            nc.scalar.dma_start(wta_sb[:], wta_t.ap().rearrange("k p m -> p k m"))
            id_sb = cpool.tile([128, 64], bf16)
            nc.scalar.dma_start(id_sb[:], id_t.ap())

            # first batch's imcol right away, before svb, split across
            # both rings: it gates the very first matmul.
            imcols = {}
            imc = ipool.tile([96, BAND * WP], bf16, tag="imcol")
            for kh, eng in ((0, nc.sync), (1, nc.scalar), (2, nc.sync)):
                eng.dma_start(
                    imc[32 * kh : 32 * kh + 32, :],
                    xs_t.ap()[0, :, kh * WP : kh * WP + BAND * WP],
                )
            imcols[0] = imc

            svb_sb = cpool.tile([128, SUPER, 2 * NBLK], bf16)
            svb_r = svb_t.ap().rearrange("p (t j) -> p t j", t=SUPER)
            for t in range(SUPER):
                nc.scalar.dma_start(svb_sb[:, t, :], svb_r[:, t, :])

            # Flat software pipeline over (b, t): conv stage at i, blend
            # stage at i-1 so the PE never waits on DVE.
            steps = [(b, t) for b in range(B) for t in range(SUPER)]
            conv_state = {}  # i -> (bc, a2)

            for i, (b, t) in enumerate(steps):
                if t == 0:
                    # prefetch next batch's imcol (current batch's was
                    # prefetched one batch ago)
                    if b + 1 < B:
                        imc = ipool.tile([96, BAND * WP], bf16, tag="imcol")
                        for kh in range(3):
                            nc.sync.dma_start(
                                imc[32 * kh : 32 * kh + 32, :],
                                xs_t.ap()[b + 1, :, kh * WP : kh * WP + BAND * WP],
                            )
                        imcols[b + 1] = imc

                imv = imcols[b].rearrange("p (h w) -> p h w", w=WP)

                # ---- conv stage for step i ----
                bc = ppool.tile([128, 2, NBLK], f32, tag="bc")
                a2 = ppool.tile([128, NBLK], f32, tag="a2")
                hl = SROWS * t
                for kw in range(3):
                    rhs0 = imv[:, hl : hl + 2, kw : kw + IMG]
                    rhs1 = imv[:, hl + 2 : hl + 4, kw : kw + IMG]
                    nc.tensor.matmul(
                        bc[:, 0, :], wtbc_sb[:, kw, :], rhs0,
                        start=(kw == 0), stop=(kw == 2),
                    )
                    nc.tensor.matmul(
                        bc[:, 1, :], wtbc_sb[:, kw, :], rhs1,
                        start=(kw == 0), stop=(kw == 2),
                    )
                for kw in range(3):
                    rhs0 = imv[:, hl : hl + 2, kw : kw + IMG]
                    rhs1 = imv[:, hl + 2 : hl + 4, kw : kw + IMG]
                    # rank-0 for both blocks: col-tiled concurrent pair
                    nc.tensor.matmul(
                        a2[0:64, :], wta_sb[:, kw, :], rhs0,
                        start=(kw == 0), stop=False, skip_group_check=True,
                    )
                    nc.tensor.matmul(
                        a2[64:128, :], wta_sb[:, kw, :], rhs1,
                        start=(kw == 0), stop=False, skip_group_check=True,
                    )
                conv_state[i] = (bc, a2)

                # ---- blend stage for step i-1 ----
                if i >= 1:
                    _blend(nc, tc, tpool, opool, conv_state, i - 1, steps,
                           svb_sb, id_sb, out_r, f32, f32r)
                    del conv_state[i - 1]

            _blend(nc, tc, tpool, opool, conv_state, len(steps) - 1, steps,
                   svb_sb, id_sb, out_r, f32, f32r)

    nc.compile()
    return nc


def _blend(nc, tc, tpool, opool, conv_state, i, steps, svb_sb, id_sb, out_r,
           f32, f32r):
    import concourse.mybir as mybir

    bf16 = mybir.dt.bfloat16
    b, t = steps[i]
    bc, a2 = conv_state[i]
    m = tpool.tile([128, 2, NBLK], bf16, tag="m")
    nc.vector.tensor_tensor(
        m[:], bc[:], svb_sb[:, t, :].rearrange("p (q j) -> p q j", q=2),
        mybir.AluOpType.mult,
    )
    # fold m's halves into a2: col-tiled concurrent pair
    nc.tensor.matmul(
        a2[0:64, :], id_sb[:], m[:, 0, :],
        start=False, stop=False, skip_group_check=True,
    )
    nc.tensor.matmul(
        a2[64:128, :], id_sb[:], m[:, 1, :],
        start=False, stop=True, skip_group_check=True,
    )
    out_sb = opool.tile([128, NBLK], f32, tag="out_sb")
    nc.scalar.copy(out_sb[:], a2[:])
    for q in range(2):
        nc.sync.dma_start(out_r[b, q, :, t, :], out_sb[64 * q : 64 * q + 64, :])


_CACHE = {}


def _get_bass():
    if "nc" not in _CACHE:
        _CACHE["nc"] = _build_bass()
    return _CACHE["nc"]


def _prep_shards(x, conv_w, kernel_weight):
    x = np.asarray(x, dtype=_F32)
    conv_w = np.asarray(conv_w, dtype=_F32)
    kernel_weight = np.asarray(kernel_weight, dtype=_F32)

    x_pad = np.pad(x, ((0, 0), (0, 0), (1, 1), (1, 1)))
    # w[kh, c, kw, (r, m)] from conv_w[(r m), c, kh, kw]
    wt = conv_w.transpose(2, 1, 3, 0).reshape(96, 3, RANK * C_OUT)
    wtbc = np.ascontiguousarray(wt[:, :, C_OUT:].transpose(1, 0, 2)).astype(_BF16)
    wta = np.ascontiguousarray(wt[:, :, :C_OUT].transpose(1, 0, 2)).astype(_BF16)
    ident = np.concatenate(
        [np.eye(64, dtype=_F32), np.eye(64, dtype=_F32)], axis=0
    ).astype(_BF16)

    in_maps = []
    for i in range(N_CORES):
        h0 = BAND * i
        shard = (
            np.ascontiguousarray(x_pad[:, :, h0 : h0 + ROWS_IN, :])
            .reshape(B, C_IN, ROWS_IN * WP)
            .astype(_BF16)
        )
        band = kernel_weight[:, h0 : h0 + BAND, :]          # [2, 32, 256]
        # svb[64r+c, (t, q, j)] = band[r, row(t, q, j)]
        arr = band.reshape(2, SUPER, 2 * NBLK)              # [r, t, (q j)]
        svb = np.broadcast_to(
            arr[:, None, :, :], (2, C_OUT, SUPER, 2 * NBLK)
        ).reshape(128, SUPER * 2 * NBLK)
        svb = np.ascontiguousarray(svb).astype(_BF16)
        in_maps.append(
            {"xs": shard, "wtbc": wtbc, "wta": wta, "svb": svb, "ident": ident}
        )
    return in_maps


def run(inputs, trace=False):
    """Run the sharded bass kernel; returns (out_full, BassKernelResults)."""
    from concourse.bass_utils import run_bass_kernel_spmd

    in_maps = _prep_shards(**inputs)
    nc = _get_bass()
    res = run_bass_kernel_spmd(
        nc, in_maps, core_ids=list(range(N_CORES)), trace=trace
    )
    out = np.empty((B, C_OUT, IMG, IMG), dtype=_F32)
    for i in range(N_CORES):
        out[:, :, BAND * i : BAND * (i + 1), :] = res.results[i]["out"]
    return out, res


def kernel(x, conv_w, kernel_weight):
    out, _ = run({"x": x, "conv_w": conv_w, "kernel_weight": kernel_weight})
    return out
